# revision 21
# baseline (speedup 1.0000x reference)
"""Trainium2 Bass kernel for block-scaled (128x128) dequant + linear:
    y[b,s,o] = sum_i x[b,s,i] * peso[o,i] * escala[o//128, i//128]

Sharding: column-parallel over 8 NeuronCores — peso/escala split along the
output dim (1536 rows each), x replicated. Each core computes its
[4096, 1536] slice of the output; the host concatenates the slices.

Device kernel (per core):
  - dequantize the peso shard into a resident fp16 W^T in SBUF, loaded in
    512-wide nb-major chunks (HWDGE/ACT ring) so matmuls can start while
    most of W is still in flight; scale+cast on DVE (tensor_scalar by the
    per-128x128-block scale)
  - stream x^T in m-slabs, cast f32->fp16 during the DMA itself (SWDGE)
  - fp16 matmuls accumulate over K=4096 in fp32 PSUM
Both matmul operands are fed K-major from host-pretransposed DRAM copies so
every DMA is contiguous (no on-device transposes).
"""

import numpy as np

# Problem shape (hardcoded per contract)
B, S, D_IN, D_OUT = 2, 2048, 4096, 12288
BLOCK = 128
N_CORES = 8
M = B * S                      # 4096 tokens
O_SHARD = D_OUT // N_CORES     # 1536 outputs per core

# Tiling
P = 128
M_SLAB = 512                   # tokens per x slab resident in SBUF (fp16)
N_TILE = 512                   # matmul moving free dim (one PSUM bank)

_compiled = None


def _build(k_dim, o_shard, m_dim):
    import concourse.mybir as mybir
    import concourse.tile as tile
    from concourse import bacc

    kb_n = k_dim // P              # k blocks
    nb_n = o_shard // N_TILE       # matmul n tiles
    ob_per_nb = N_TILE // P        # scale blocks per n tile (4)
    slab_n = m_dim // M_SLAB
    mt_n = M_SLAB // P             # m tiles per slab
    k_chunk = min(8, kb_n)         # k blocks per x DMA chunk
    chunk_n = kb_n // k_chunk

    f32 = mybir.dt.float32
    f16 = mybir.dt.float16

    nc = bacc.Bacc("TRN2", target_bir_lowering=False, debug=False,
                   enable_asserts=False)
    xT = nc.dram_tensor("xT", [k_dim, m_dim], f32, kind="ExternalInput").ap()
    wT = nc.dram_tensor("wT", [k_dim, o_shard], f32, kind="ExternalInput").ap()
    esc = nc.dram_tensor("esc", [P, kb_n * nb_n * ob_per_nb], f32,
                         kind="ExternalInput").ap()
    out = nc.dram_tensor("out", [m_dim, o_shard], f32, kind="ExternalOutput").ap()

    with tile.TileContext(nc) as tc:
        with (
            tc.tile_pool(name="wres", bufs=1) as wres_pool,
            tc.tile_pool(name="escp", bufs=1) as esc_pool,
            tc.tile_pool(name="wstage", bufs=4) as wstage_pool,
            tc.tile_pool(name="xbf", bufs=2) as xbf_pool,
            tc.tile_pool(name="outst", bufs=4) as out_pool,
            tc.tile_pool(name="psum", bufs=6, space="PSUM") as psum_pool,
        ):
            esc_sb = esc_pool.tile([P, kb_n * nb_n * ob_per_nb], f32)
            nc.sync.dma_start(out=esc_sb[:], in_=esc[:])

            wres = [wres_pool.tile([P, o_shard], f16, tag=f"wres{kb}",
                                   name=f"wres{kb}")
                    for kb in range(kb_n)]

            def chunk_layout(ms):
                # slab 0 front-loads two small chunks so the first matmul
                # group can start as soon as ~1 MB of x has landed; steady
                # slabs use efficient 2 MB transfers
                if ms == 0 and kb_n > k_chunk:
                    return [k_chunk // 2, k_chunk // 2] + \
                           [k_chunk] * (chunk_n - 1)
                return [k_chunk] * chunk_n

            def emit_x_slab(ms):
                # one tile per k-chunk so matmuls only wait on the chunk
                # they actually read, not the whole 8 MB slab
                chunks = []
                kb2chunk = {}
                m0 = ms * M_SLAB
                kb0 = 0
                for c, sz in enumerate(chunk_layout(ms)):
                    xc = xbf_pool.tile([P, sz, M_SLAB], f16,
                                       tag=f"xbf{c}", name=f"xbf{ms}_{c}",
                                       bufs=1 if c >= chunk_n else None)
                    src = xT[kb0 * P:(kb0 + sz) * P, m0:m0 + M_SLAB]
                    nc.gpsimd.dma_start(
                        out=xc[:],
                        in_=src.rearrange("(kb p) m -> p kb m", p=P),
                    )
                    chunks.append(xc)
                    for kk in range(sz):
                        kb2chunk[kb0 + kk] = (c, kk)
                    kb0 += sz
                return chunks, kb2chunk

            def emit_w_prep(nb):
                # load + dequantize W^T[:, nb*512:(nb+1)*512] for all k blocks
                for kb in range(kb_n):
                    w_f32 = wstage_pool.tile([P, N_TILE], f32, tag="wstage",
                                             name=f"wstg{nb}_{kb}")
                    nc.scalar.dma_start(
                        out=w_f32[:],
                        in_=wT[kb * P:(kb + 1) * P,
                               nb * N_TILE:(nb + 1) * N_TILE],
                    )
                    for j in range(ob_per_nb):
                        ob = nb * ob_per_nb + j
                        nc.vector.tensor_scalar_mul(
                            wres[kb][:, nb * N_TILE + j * P:
                                     nb * N_TILE + (j + 1) * P],
                            w_f32[:, j * P:(j + 1) * P],
                            esc_sb[:, kb * (nb_n * ob_per_nb) + ob:
                                   kb * (nb_n * ob_per_nb) + ob + 1],
                        )

            def emit_group(x_slab, ms, nb, mt):
                x_bf, kb2chunk = x_slab
                ps = psum_pool.tile([P, N_TILE], f32, tag="psum",
                                    name=f"ps{ms}_{nb}_{mt}")
                for kb in range(kb_n):
                    c, kk = kb2chunk[kb]
                    nc.tensor.matmul(
                        ps[:],
                        x_bf[c][:, kk, mt * P:(mt + 1) * P],
                        wres[kb][:, nb * N_TILE:(nb + 1) * N_TILE],
                        start=(kb == 0),
                        stop=(kb == kb_n - 1),
                    )
                o_sb = out_pool.tile([P, N_TILE], f32, tag="outst",
                                     name=f"osb{ms}_{nb}_{mt}")
                nc.vector.tensor_copy(out=o_sb[:], in_=ps[:])
                row0 = ms * M_SLAB + mt * P
                nc.sync.dma_start(
                    out=out[row0:row0 + P, nb * N_TILE:(nb + 1) * N_TILE],
                    in_=o_sb[:],
                )

            x_cur = emit_x_slab(0)
            emit_w_prep(0)
            x_next = emit_x_slab(1) if slab_n > 1 else None
            x_after = None
            for ms in range(slab_n):
                for nb in range(nb_n):
                    for mt in range(mt_n):
                        emit_group(x_cur, ms, nb, mt)
                    if ms == 0 and nb + 1 < nb_n:
                        emit_w_prep(nb + 1)
                    if nb == 0 and ms + 2 < slab_n:
                        x_after = emit_x_slab(ms + 2)
                x_cur, x_next = x_next, None
                if ms + 2 < slab_n:
                    x_next = x_after

    nc.compile()
    return nc


def _prep_inputs(x, peso, escala):
    xT = np.ascontiguousarray(x.reshape(M, D_IN).T)           # [K, M]
    pT = peso.T                                               # [K, O] view
    in_maps = []
    for i in range(N_CORES):
        o0 = i * O_SHARD
        wT_i = np.ascontiguousarray(pT[:, o0:o0 + O_SHARD])   # [K, 1536]
        esc_sh = escala[i * (O_SHARD // P):(i + 1) * (O_SHARD // P), :]
        # flat[j], j = kb * ob_n + ob  ->  escala_shard[ob, kb]
        esc_flat = np.ascontiguousarray(esc_sh.T).reshape(-1)
        esc_i = np.ascontiguousarray(
            np.broadcast_to(esc_flat, (P, esc_flat.size)))
        in_maps.append({"xT": xT, "wT": wT_i, "esc": esc_i})
    return in_maps


def kernel(x, peso, escala):
    from concourse import bass_utils

    global _compiled
    if _compiled is None:
        _compiled = _build(D_IN, O_SHARD, M)

    in_maps = _prep_inputs(np.asarray(x, dtype=np.float32),
                           np.asarray(peso, dtype=np.float32),
                           np.asarray(escala, dtype=np.float32))
    res = bass_utils.run_bass_kernel_spmd(_compiled, in_maps,
                                          list(range(N_CORES)))
    global last_result
    last_result = res
    shards = [res.results[i]["out"] for i in range(N_CORES)]
    y = np.concatenate(shards, axis=1).reshape(B, S, D_OUT)
    return np.ascontiguousarray(y)


# revision 22
# speedup vs baseline: 1.0607x; 1.0607x over previous
"""Trainium2 Bass kernel for block-scaled (128x128) dequant + linear:
    y[b,s,o] = sum_i x[b,s,i] * peso[o,i] * escala[o//128, i//128]

Sharding: column-parallel over 8 NeuronCores — peso/escala split along the
output dim (1536 rows each), x replicated. Each core computes its
[4096, 1536] slice of the output; the host concatenates the slices.

Device kernel (per core):
  - dequantize the peso shard into a resident fp16 W^T in SBUF, loaded in
    512-wide nb-major chunks (HWDGE/ACT ring) so matmuls can start while
    most of W is still in flight; scale+cast on DVE (tensor_scalar by the
    per-128x128-block scale)
  - stream x^T in m-slabs, cast f32->fp16 during the DMA itself (SWDGE)
  - fp16 matmuls accumulate over K=4096 in fp32 PSUM
Both matmul operands are fed K-major from host-pretransposed DRAM copies so
every DMA is contiguous (no on-device transposes).
"""

import numpy as np

# Problem shape (hardcoded per contract)
B, S, D_IN, D_OUT = 2, 2048, 4096, 12288
BLOCK = 128
N_CORES = 8
M = B * S                      # 4096 tokens
O_SHARD = D_OUT // N_CORES     # 1536 outputs per core

# Tiling
P = 128
M_SLAB = 512                   # tokens per x slab resident in SBUF (fp16)
N_TILE = 512                   # matmul moving free dim (one PSUM bank)

_compiled = None


def _build(k_dim, o_shard, m_dim):
    import concourse.mybir as mybir
    import concourse.tile as tile
    from concourse import bacc

    kb_n = k_dim // P              # k blocks
    nb_n = o_shard // N_TILE       # matmul n tiles
    ob_per_nb = N_TILE // P        # scale blocks per n tile (4)
    slab_n = m_dim // M_SLAB
    mt_n = M_SLAB // P             # m tiles per slab
    k_chunk = min(8, kb_n)         # k blocks per x DMA chunk
    chunk_n = kb_n // k_chunk

    f32 = mybir.dt.float32
    f16 = mybir.dt.float16

    nc = bacc.Bacc("TRN2", target_bir_lowering=False, debug=False,
                   enable_asserts=False)
    xT = nc.dram_tensor("xT", [k_dim, m_dim], f32, kind="ExternalInput").ap()
    wT = nc.dram_tensor("wT", [k_dim, o_shard], f32, kind="ExternalInput").ap()
    esc = nc.dram_tensor("esc", [P, kb_n * nb_n * ob_per_nb], f32,
                         kind="ExternalInput").ap()
    out = nc.dram_tensor("out", [m_dim, o_shard], f32, kind="ExternalOutput").ap()

    with tile.TileContext(nc) as tc:
        with (
            tc.tile_pool(name="wres", bufs=1) as wres_pool,
            tc.tile_pool(name="escp", bufs=1) as esc_pool,
            tc.tile_pool(name="wstage", bufs=4) as wstage_pool,
            tc.tile_pool(name="xbf", bufs=2) as xbf_pool,
            tc.tile_pool(name="outst", bufs=4) as out_pool,
            tc.tile_pool(name="psum", bufs=6, space="PSUM") as psum_pool,
        ):
            esc_sb = esc_pool.tile([P, kb_n * nb_n * ob_per_nb], f32)
            nc.sync.dma_start(out=esc_sb[:], in_=esc[:])

            wres = [wres_pool.tile([P, o_shard], f16, tag=f"wres{kb}",
                                   name=f"wres{kb}")
                    for kb in range(kb_n)]

            def chunk_layout(ms):
                # slab 0 front-loads two small chunks so the first matmul
                # group can start as soon as ~1 MB of x has landed; steady
                # slabs use efficient 2 MB transfers
                if ms == 0 and kb_n > k_chunk:
                    return [k_chunk // 2, k_chunk // 2] + \
                           [k_chunk] * (chunk_n - 1)
                return [k_chunk] * chunk_n

            def emit_x_slab(ms):
                # one tile per k-chunk so matmuls only wait on the chunk
                # they actually read, not the whole 8 MB slab
                chunks = []
                kb2chunk = {}
                m0 = ms * M_SLAB
                kb0 = 0
                for c, sz in enumerate(chunk_layout(ms)):
                    xc = xbf_pool.tile([P, sz, M_SLAB], f16,
                                       tag=f"xbf{c}", name=f"xbf{ms}_{c}",
                                       bufs=1 if c >= chunk_n else None)
                    src = xT[kb0 * P:(kb0 + sz) * P, m0:m0 + M_SLAB]
                    nc.gpsimd.dma_start(
                        out=xc[:],
                        in_=src.rearrange("(kb p) m -> p kb m", p=P),
                    )
                    chunks.append(xc)
                    for kk in range(sz):
                        kb2chunk[kb0 + kk] = (c, kk)
                    kb0 += sz
                return chunks, kb2chunk

            def emit_w_prep(nb):
                # load + dequantize W^T[:, nb*512:(nb+1)*512] for all k blocks
                for kb in range(kb_n):
                    w_f32 = wstage_pool.tile([P, N_TILE], f32, tag="wstage",
                                             name=f"wstg{nb}_{kb}")
                    nc.scalar.dma_start(
                        out=w_f32[:],
                        in_=wT[kb * P:(kb + 1) * P,
                               nb * N_TILE:(nb + 1) * N_TILE],
                    )
                    for j in range(ob_per_nb):
                        ob = nb * ob_per_nb + j
                        nc.vector.tensor_scalar_mul(
                            wres[kb][:, nb * N_TILE + j * P:
                                     nb * N_TILE + (j + 1) * P],
                            w_f32[:, j * P:(j + 1) * P],
                            esc_sb[:, kb * (nb_n * ob_per_nb) + ob:
                                   kb * (nb_n * ob_per_nb) + ob + 1],
                        )

            def emit_group(x_slab, ms, nb, mt):
                x_bf, kb2chunk = x_slab
                ps = psum_pool.tile([P, N_TILE], f32, tag="psum",
                                    name=f"ps{ms}_{nb}_{mt}")
                for kb in range(kb_n):
                    c, kk = kb2chunk[kb]
                    nc.tensor.matmul(
                        ps[:],
                        x_bf[c][:, kk, mt * P:(mt + 1) * P],
                        wres[kb][:, nb * N_TILE:(nb + 1) * N_TILE],
                        start=(kb == 0),
                        stop=(kb == kb_n - 1),
                    )
                o_sb = out_pool.tile([P, N_TILE], f32, tag="outst",
                                     name=f"osb{ms}_{nb}_{mt}")
                nc.vector.tensor_copy(out=o_sb[:], in_=ps[:])
                row0 = ms * M_SLAB + mt * P
                nc.sync.dma_start(
                    out=out[row0:row0 + P, nb * N_TILE:(nb + 1) * N_TILE],
                    in_=o_sb[:],
                )

            def emit_block(x_slab, ms, nb):
                for mt in range(mt_n):
                    emit_group(x_slab, ms, nb, mt)

            x0 = emit_x_slab(0)
            emit_w_prep(0)
            if slab_n == 1:
                for nb in range(nb_n):
                    emit_block(x0, 0, nb)
                    if nb + 1 < nb_n:
                        emit_w_prep(nb + 1)
            else:
                # W-load phase covers slabs 0 and 1 W-slice-major: nb0 on
                # both slabs runs while the nb1/nb2 weight slices are still
                # in flight, so the PE has 2x the work per delivered W byte
                # and the DMA-bound ramp stays stall-free
                x1 = emit_x_slab(1)
                emit_block(x0, 0, 0)
                for nb in range(1, nb_n):
                    emit_w_prep(nb)
                emit_block(x1, 1, 0)
                for nb in range(1, nb_n):
                    emit_block(x0, 0, nb)
                x_next = emit_x_slab(2) if slab_n > 2 else None
                for nb in range(1, nb_n):
                    emit_block(x1, 1, nb)
                x_cur = x_next
                for ms in range(2, slab_n):
                    for nb in range(nb_n):
                        emit_block(x_cur, ms, nb)
                        if nb == 0 and ms + 1 < slab_n:
                            x_next = emit_x_slab(ms + 1)
                    x_cur = x_next

    nc.compile()
    return nc


def _prep_inputs(x, peso, escala):
    xT = np.ascontiguousarray(x.reshape(M, D_IN).T)           # [K, M]
    pT = peso.T                                               # [K, O] view
    in_maps = []
    for i in range(N_CORES):
        o0 = i * O_SHARD
        wT_i = np.ascontiguousarray(pT[:, o0:o0 + O_SHARD])   # [K, 1536]
        esc_sh = escala[i * (O_SHARD // P):(i + 1) * (O_SHARD // P), :]
        # flat[j], j = kb * ob_n + ob  ->  escala_shard[ob, kb]
        esc_flat = np.ascontiguousarray(esc_sh.T).reshape(-1)
        esc_i = np.ascontiguousarray(
            np.broadcast_to(esc_flat, (P, esc_flat.size)))
        in_maps.append({"xT": xT, "wT": wT_i, "esc": esc_i})
    return in_maps


def kernel(x, peso, escala):
    from concourse import bass_utils

    global _compiled
    if _compiled is None:
        _compiled = _build(D_IN, O_SHARD, M)

    in_maps = _prep_inputs(np.asarray(x, dtype=np.float32),
                           np.asarray(peso, dtype=np.float32),
                           np.asarray(escala, dtype=np.float32))
    res = bass_utils.run_bass_kernel_spmd(_compiled, in_maps,
                                          list(range(N_CORES)))
    global last_result
    last_result = res
    shards = [res.results[i]["out"] for i in range(N_CORES)]
    y = np.concatenate(shards, axis=1).reshape(B, S, D_OUT)
    return np.ascontiguousarray(y)
